# revision 28
# baseline (speedup 1.0000x reference)
"""Trainium2 Bass kernel for gated-adapter attention (Llama-Adapter style).

Sharding: 2 data-parallel groups of 4 cores (batch dim); within a group the 32
heads are tensor-parallel (8 heads/core).  Each core computes QKV + RoPE +
causal flash-style attention (transposed scores) + gated adapter cross
attention for its heads, AllGathers the per-head attention output across its
group of 4, then computes a column shard of the wo projection.  The host
reassembles the full [2, 2048, 4096] output from the 8 per-core shards.

Structure (v2):
 - Phase A: two head-half passes; each pass computes Q,K,V for 4 heads in one
   dense matmul stream (3 N=512 matmuls per (tchunk, m) sharing the x
   stationary), so the PE never idles at projection boundaries and the HAM
   clock stays warm.  RoPE + per-head PE transposes produce qT/kT layouts.
 - Phase B: attention blocks and wo chunks interleaved in one scope so wo
   matmuls fill softmax-latency gaps; AllGather(qb) overlaps attention(qb-1).

All TensorEngine tensors are fp16 (1 cycle/row, ~0.05% rounding); PSUM
accumulation is fp32; softmax sums/reciprocals are fp32.
"""

import math
import os
import sys

import numpy as np

for _p in ("/opt/trn_rl_repo",):
    if os.path.isdir(_p) and _p not in sys.path:
        sys.path.insert(0, _p)

import ml_dtypes  # noqa: E402

import concourse.bass as bass  # noqa: E402
import concourse.mybir as mybir  # noqa: E402
import concourse.tile as tile  # noqa: E402
from concourse import bacc  # noqa: E402

FP16 = np.float16
F16 = mybir.dt.float16
F32 = mybir.dt.float32

DIM = 4096
S = 2048
B = 2
H = 32
HD = 128
ALEN = 10

NCORES = 8
CPG = 4          # cores per group (group = one batch element)
HPC = 8          # heads per core
OC = HPC * HD    # 1024 output dims per core for q/k/v and for wo columns
HHP = 4          # heads per pass
OCH = HHP * HD   # 512 output dims per pass

TCN = 17         # t-chunks of 128: 16 real + 1 adapter/pad chunk
TAUG = TCN * 128  # 2176
NM = DIM // 128   # 32 contraction chunks
QB = 4           # query blocks
QW = 512         # query block width
SCALE = 1.0 / math.sqrt(HD)

REPLICA_GROUPS = [[0, 1, 2, 3], [4, 5, 6, 7]]

Exp = mybir.ActivationFunctionType.Exp
Copy = mybir.ActivationFunctionType.Copy


def _alu():
    from concourse.alu_op_type import AluOpType
    return AluOpType


def build_graph():
    nc = bacc.Bacc(
        "TRN2",
        target_bir_lowering=False,
        debug=False,
        num_devices=NCORES,
    )

    # ---- external I/O ------------------------------------------------------
    xT = nc.dram_tensor("xT", [DIM, TAUG], F16, kind="ExternalInput")
    wqT = nc.dram_tensor("wqT", [DIM, OC], F16, kind="ExternalInput")
    wkT = nc.dram_tensor("wkT", [DIM, OC], F16, kind="ExternalInput")
    wvT = nc.dram_tensor("wvT", [DIM, OC], F16, kind="ExternalInput")
    woT = nc.dram_tensor("woT", [DIM, OC], F16, kind="ExternalInput")
    cosP = nc.dram_tensor("cosP", [S, 64], F16, kind="ExternalInput")
    sinP = nc.dram_tensor("sinP", [S, 64], F16, kind="ExternalInput")
    maskmul = nc.dram_tensor("maskmul", [4, 128, QW], F16, kind="ExternalInput")
    gates = nc.dram_tensor("gates", [16, HPC], F32, kind="ExternalInput")
    eye = nc.dram_tensor("eye", [128, 128], F16, kind="ExternalInput")
    out_ext = nc.dram_tensor("out", [OC, S], F16, kind="ExternalOutput")

    op = _alu()

    with tile.TileContext(nc) as tc:
        with (
            tc.tile_pool(name="persist", bufs=1) as persist,
            tc.tile_pool(name="dram", bufs=1, space="DRAM") as dpool,
        ):
            kT = persist.tile([128, HPC * S], F16, tag="kT")     # [d, h*t]
            vsb = persist.tile([128, TCN * OC], F16, tag="vsb")  # [t, tc*o]
            akT = persist.tile([128, HPC * 16], F16, tag="akT")  # [d, h*16]
            avg = persist.tile([16, HPC * HD], F16, tag="avg")   # [a, h*d]
            ones = persist.tile([128, 128], F16, tag="ones")
            eyesb = persist.tile([128, 128], F16, tag="eyesb")
            gatesb = persist.tile([16, HPC], F32, tag="gatesb")
            cossb = persist.tile([128, 16 * 64], F16, tag="cossb")
            sinsb = persist.tile([128, 16 * 64], F16, tag="sinsb")
            negC = persist.tile([128, 1], F32, tag="negC")
            # diagonal-block exp masks are the same for every query block
            # (k0 - q0 is fixed per dk), so 4 tiles cover all of them
            mask4 = persist.tile([128, 4 * QW], F16, tag="mask4")

            nc.gpsimd.memset(ones[:], 1.0)
            nc.gpsimd.memset(negC[:], -9.0)
            nc.sync.dma_start(eyesb[:], eye[:])
            nc.sync.dma_start(gatesb[:], gates[:])
            nc.sync.dma_start(
                mask4[:].rearrange("p (d q) -> p d q", d=4),
                maskmul[:].rearrange("d p q -> p d q"),
            )
            nc.sync.dma_start(
                cossb[:].rearrange("p (c d) -> p c d", c=16),
                cosP[:].rearrange("(c p) d -> p c d", p=128),
            )
            nc.sync.dma_start(
                sinsb[:].rearrange("p (c d) -> p c d", c=16),
                sinP[:].rearrange("(c p) d -> p c d", p=128),
            )

            qTd = [dpool.tile([128, HPC * QW], F16, tag=f"qTd{q}",
                              name=f"qTd{q}")
                   for q in range(QB)]
            agin = [dpool.tile([OC, QW], F16, tag=f"agin{q}", name=f"agin{q}")
                    for q in range(QB)]
            agouth = [[dpool.tile([CPG * 512, QW], F16, tag=f"ago{q}_{hf}",
                                  name=f"ago{q}_{hf}")
                       for hf in range(2)]
                      for q in range(QB)]
            # qb=0 gathers per-head so the tail never waits a big collective
            ago0 = [dpool.tile([CPG * 128, QW], F16, tag=f"ago0h{h}",
                               name=f"ago0h{h}")
                    for h in range(HPC)]

            # ============ Phase A: QKV + RoPE, two head-half passes ========
            with (
                tc.tile_pool(name="wres", bufs=96) as wpool,
                tc.tile_pool(name="xin", bufs=30) as xpool,
                tc.tile_pool(name="xin8", bufs=6) as xpool8,
                tc.tile_pool(name="asm", bufs=5) as apool,
                tc.tile_pool(name="rot", bufs=5) as rpool,
                tc.tile_pool(name="qst", bufs=3) as qstpool,
                tc.tile_pool(name="psq", bufs=2, space="PSUM") as psqp,
                tc.tile_pool(name="psk", bufs=2, space="PSUM") as pskp,
                tc.tile_pool(name="psv", bufs=2, space="PSUM") as psvp,
                tc.tile_pool(name="pstr", bufs=2, space="PSUM") as ptpool,
            ):
                pending = []   # deferred PE transposes + copies

                def rope(proj, hp, tch, asmt):
                    """asmt: [128 tokens, OCH] = 4 heads, deinterleaved rope
                    pairs.  Rotate now (DVE); defer the PE transposes so the
                    PE queue never stalls on the rope pipeline."""
                    a3 = asmt[:].rearrange("p (h d) -> p h d", h=HHP)[:, :, 0:64]
                    b3 = asmt[:].rearrange("p (h d) -> p h d", h=HHP)[:, :, 64:128]
                    cos1 = cossb[:].rearrange(
                        "p (c o d) -> p c o d", c=16, o=1)[:, tch]
                    sin1 = sinsb[:].rearrange(
                        "p (c o d) -> p c o d", c=16, o=1)[:, tch]
                    cos3, _ = bass.broadcast_tensor_aps(cos1, a3)
                    sin3, _ = bass.broadcast_tensor_aps(sin1, a3)
                    rot = rpool.tile([128, OCH], F16, tag="rot",
                                     name=f"rot{proj}{hp}_{tch}")
                    ra = rot[:].rearrange("p (h d) -> p h d", h=HHP)[:, :, 0:64]
                    rb = rot[:].rearrange("p (h d) -> p h d", h=HHP)[:, :, 64:128]
                    t1 = rpool.tile([128, HHP * 64], F16, tag="rt1",
                                    name=f"rt1{proj}{hp}_{tch}")
                    t13 = t1[:].rearrange("p (h d) -> p h d", h=HHP)
                    t2 = rpool.tile([128, HHP * 64], F16, tag="rt2",
                                    name=f"rt2{proj}{hp}_{tch}")
                    t23 = t2[:].rearrange("p (h d) -> p h d", h=HHP)
                    nc.vector.tensor_tensor(t13, a3, cos3, op.mult)
                    nc.vector.tensor_tensor(t23, b3, sin3, op.mult)
                    nc.vector.tensor_tensor(ra, t13, t23, op.subtract)
                    nc.vector.tensor_tensor(t13, a3, sin3, op.mult)
                    nc.vector.tensor_tensor(t23, b3, cos3, op.mult)
                    nc.vector.tensor_tensor(rb, t13, t23, op.add)
                    pending.append((proj, hp, tch, rot))

                def flush_pending():
                    for proj, hp, tch, rot in pending:
                        ptr = ptpool.tile([128, OCH], F16, tag="pstr",
                                          name=f"ptr{proj}{hp}_{tch}")
                        for hl in range(HHP):
                            nc.tensor.transpose(
                                ptr[:, hl * 128:(hl + 1) * 128],
                                rot[:, hl * 128:(hl + 1) * 128],
                                eyesb[:],
                            )
                        h0 = hp * HHP
                        if proj == "ak":
                            nc.scalar.activation(
                                akT[:].rearrange(
                                    "p (h a) -> p h a",
                                    h=HPC)[:, h0:h0 + HHP, :],
                                ptr[:].rearrange(
                                    "p (h t) -> p h t", h=HHP)[:, :, 0:16],
                                Copy,
                            )
                        elif proj == "k":
                            nc.scalar.activation(
                                kT[:].rearrange(
                                    "p (h t) -> p h t",
                                    h=HPC)[:, h0:h0 + HHP,
                                           tch * 128:(tch + 1) * 128],
                                ptr[:].rearrange("p (h d) -> p h d", h=HHP),
                                Copy,
                            )
                        else:
                            qst = qstpool.tile([128, OCH], F16, tag="qst",
                                               name=f"qst{hp}_{tch}")
                            nc.scalar.activation(qst[:], ptr[:], Copy)
                            qb_of, tl = divmod(tch, 4)
                            nc.scalar.dma_start(
                                qTd[qb_of][:].rearrange(
                                    "p (h t) -> p h t",
                                    h=HPC)[:, h0:h0 + HHP,
                                           tl * 128:(tl + 1) * 128],
                                qst[:].rearrange("p (h d) -> p h d", h=HHP),
                            )
                    pending.clear()

                for hp in range(2):
                    osl = slice(hp * OCH, (hp + 1) * OCH)
                    wres = {"q": [], "k": [], "v": []}

                    # m-major emission so DMA arrival matches the (q,k,v)
                    # per-m consumption order; only 8 groups up front so the
                    # first x tiles aren't queued behind 12MB of weights
                    def load_w_group(m, hp=hp, osl=osl, wres=wres):
                        for proj, wsrc in (("q", wqT), ("k", wkT),
                                           ("v", wvT)):
                            wt = wpool.tile([128, OCH], F16, tag="wres",
                                            name=f"w{proj}{hp}_{m}")
                            nc.sync.dma_start(
                                wt[:], wsrc[m * 128:(m + 1) * 128, osl])
                            wres[proj].append(wt)

                    for m in range(8):
                        load_w_group(m)
                    # x is staged in 2-chunk slabs: tch 16 gets its own
                    # [128,128] tile; even tch prefetches the slab covering
                    # (tch-2, tch-1) a full t-chunk before first use so the
                    # PE never waits on the x feed.
                    xslab = {}
                    # descending: qb=3's q/k chunks finish first so the
                    # first attention block's inputs are ready earliest
                    for tch in range(TCN - 1, -1, -1):
                        psq = None
                        if tch < 16:
                            psq = psqp.tile([128, OCH], F32, tag="psq",
                                            name=f"psq{hp}_{tch}")
                        psk = pskp.tile([128, OCH], F32, tag="psk",
                                        name=f"psk{hp}_{tch}")
                        psv = psvp.tile([128, OCH], F32, tag="psv",
                                        name=f"psv{hp}_{tch}")
                        for m in range(NM):
                            if tch == TCN - 1 and m + 8 < NM:
                                load_w_group(m + 8)
                            if tch == 16:
                                t = xpool8.tile([128, 128], F16, tag="xin8",
                                                name=f"x{hp}_{m}_s8")
                                nc.sync.dma_start(
                                    t[:], xT[m * 128:(m + 1) * 128,
                                             2048:2176])
                                xslab[m] = t
                            if tch == 16:
                                xm_ap = xslab[m][:]
                            else:
                                off = tch % 2
                                xm_ap = xslab[m][:, off * 128:(off + 1) * 128]
                            st = (m == 0)
                            sp = (m == NM - 1)
                            if tch < 16:
                                nc.tensor.matmul(
                                    psq[:], lhsT=xm_ap, rhs=wres["q"][m][:],
                                    start=st, stop=sp)
                            nc.tensor.matmul(
                                psk[:], lhsT=xm_ap, rhs=wres["k"][m][:],
                                start=st, stop=sp)
                            nc.tensor.matmul(
                                psv[:], lhsT=xm_ap, rhs=wres["v"][m][:],
                                start=st, stop=sp)
                            if tch % 2 == 0 and tch >= 2:
                                t = xpool.tile([128, 256], F16, tag="xin",
                                               name=f"x{hp}_{m}_s{tch//2-1}")
                                nc.sync.dma_start(
                                    t[:], xT[m * 128:(m + 1) * 128,
                                             (tch - 2) * 128:tch * 128])
                                xslab[m] = t
                        # ---- V: evacuate straight to vsb ------------------
                        nc.vector.tensor_copy(
                            vsb[:, tch * OC + hp * OCH:
                                tch * OC + (hp + 1) * OCH], psv[:])
                        if tch == 16:
                            for hl in range(HHP):
                                h = hp * HHP + hl
                                nc.vector.tensor_scalar(
                                    avg[0:10, h * HD:(h + 1) * HD],
                                    vsb[0:10, 16 * OC + h * HD:
                                        16 * OC + (h + 1) * HD],
                                    gatesb[0:10, h:h + 1],
                                    None,
                                    op.mult,
                                )
                        # ---- K (+ Q): evacuate, rope; transposes deferred -
                        asmk = apool.tile([128, OCH], F16, tag="asm",
                                          name=f"ask{hp}_{tch}")
                        nc.scalar.activation(asmk[:], psk[:], Copy)
                        asmq = None
                        if tch < 16:
                            asmq = apool.tile([128, OCH], F16, tag="asm",
                                              name=f"asq{hp}_{tch}")
                            nc.scalar.activation(asmq[:], psq[:], Copy)
                        flush_pending()
                        if tch < 16:
                            rope("k", hp, tch, asmk)
                            rope("q", hp, tch, asmq)
                        else:
                            pending.append(("ak", hp, tch, asmk))
                flush_pending()

            # ====== Phase B: attention + AllGather + wo, interleaved =======
            from contextlib import ExitStack
            _es = ExitStack()
            with _es:
                P = lambda **kw: _es.enter_context(tc.tile_pool(**kw))
                prpool = P(name="probs", bufs=22)
                qldpool = P(name="qld", bufs=2)
                smpool = P(name="small", bufs=3)
                rcpool = P(name="rec", bufs=2)
                ctpool = P(name="ctmp", bufs=2)
                w2pool = P(name="w2", bufs=8)
                agpool = P(name="agsb", bufs=4)
                ostpool = P(name="ost", bufs=2)
                pscp = P(name="psc", bufs=4, space="PSUM")
                ppvp = P(name="ppv", bufs=2, space="PSUM")
                psmp = P(name="psums", bufs=1, space="PSUM")
                pwop = P(name="pwo", bufs=1, space="PSUM")

                qslabs = {}

                def load_qslab(qb):
                    qt = qldpool.tile([128, HPC * QW], F16, tag="qld",
                                      name=f"qs{qb}")
                    nc.sync.dma_start(qt[:], qTd[qb][:])
                    qslabs[qb] = qt

                load_qslab(QB - 1)

                # gathered-activation staging: each collective half is
                # reloaded as four 512KB pieces so no single DMA blocks the
                # Sync queue long enough to starve the w2 feed
                ag_slabs = {}
                agq0_slabs = {}

                def stage_ag_half(qb, hf):
                    # on the gpsimd queue: never blocks the w2/q feeds,
                    # and is only emitted at points where its gather has
                    # provably completed
                    slab = agpool.tile([128, 16 * QW], F16, tag="agsb",
                                       name=f"ags{qb}_{hf}")
                    ag_slabs.setdefault(qb, {})[hf] = slab
                    nc.gpsimd.dma_start(
                        slab[:].rearrange("p (b q) -> p b q", b=16),
                        agouth[qb][hf][:].rearrange(
                            "(b p) q -> p b q", p=128),
                    )

                def stage_agq0(h):
                    sl = h // 4
                    if sl not in agq0_slabs:
                        agq0_slabs[sl] = agpool.tile(
                            [128, 16 * QW], F16, tag="agsb",
                            name=f"agq0s{sl}")
                    slab = agq0_slabs[sl]
                    hl = h % 4
                    nc.gpsimd.dma_start(
                        slab[:, hl * 4 * QW:(hl + 1) * 4 * QW].rearrange(
                            "p (c q) -> p c q", c=CPG),
                        ago0[h][:].rearrange("(c p) q -> p c q", p=128),
                    )

                pend = {}

                def scores_part(qb, h):
                    """scores + exp (+ adapter scores) for one head.  Runs
                    one head ahead of pv_part so the Act engine's exp
                    latency hides under the previous head's pv stream."""
                    kk = (qb + 1) * 4  # causal: k chunks 0..kk-1
                    if h == 0 and qb > 0:
                        load_qslab(qb - 1)  # prefetch next block's q
                    q_ap = qslabs[qb][:, h * QW:(h + 1) * QW]
                    pbs = []
                    for kc in range(kk):
                        sc = pscp.tile([128, QW], F32, tag="sc",
                                       name=f"sc{qb}_{h}_{kc}")
                        nc.tensor.matmul(
                            sc[:],
                            lhsT=kT[:, h * S + kc * 128:
                                    h * S + (kc + 1) * 128],
                            rhs=q_ap,
                            start=True, stop=True,
                        )
                        pb = prpool.tile([128, QW], F16, tag="probs",
                                         name=f"pb{qb}_{h}_{kc}")
                        nc.scalar.activation(pb[:], sc[:], Exp,
                                             bias=negC[:, 0:1],
                                             scale=SCALE)
                        if kc >= qb * 4:
                            dk = kc - qb * 4
                            nc.vector.tensor_tensor(
                                pb[:], pb[:],
                                mask4[:, dk * QW:(dk + 1) * QW], op.mult)
                        pbs.append(pb)
                    asc = pscp.tile([10, QW], F32, tag="sc",
                                    name=f"asc{qb}_{h}")
                    nc.tensor.matmul(
                        asc[:], lhsT=akT[:, h * 16:h * 16 + 10],
                        rhs=q_ap, start=True, stop=True)
                    apb = smpool.tile([10, QW], F16, tag="aprobs",
                                      name=f"apb{qb}_{h}")
                    nc.scalar.activation(apb[:], asc[:], Exp,
                                         bias=negC[0:10, 0:1],
                                         scale=SCALE)
                    pend[(qb, h)] = (pbs, apb)

                def pv_part(qb, h):
                    kk = (qb + 1) * 4
                    pbs, apb = pend.pop((qb, h))
                    # sums matmuls use a full ones[128,128] stationary, so
                    # the row-sum lands replicated on all 128 partitions --
                    # no gpsimd partition_broadcast needed downstream.
                    smt = psmp.tile([128, QW], F32, tag="sums",
                                    name=f"sums{qb}_{h}")
                    sums = smt[:]
                    pv = ppvp.tile([128, QW], F32, tag="pv",
                                   name=f"pv{qb}_{h}")
                    for i in range(kk):
                        nc.tensor.matmul(
                            sums, lhsT=ones[:], rhs=pbs[i][:],
                            start=(i == 0), stop=(i == kk - 1),
                        )
                        nc.tensor.matmul(
                            pv[:],
                            lhsT=vsb[:, i * OC + h * HD:
                                     i * OC + (h + 1) * HD],
                            rhs=pbs[i][:],
                            start=(i == 0), stop=(i == kk - 1),
                        )
                    sat = pscp.tile([128, QW], F32, tag="sc",
                                    name=f"sumA{qb}_{h}")
                    sumA = sat[:]
                    nc.tensor.matmul(
                        sumA, lhsT=ones[0:10, :], rhs=apb[:],
                        start=True, stop=True)
                    apv = ppvp.tile([128, QW], F32, tag="pv",
                                    name=f"apv{qb}_{h}")
                    nc.tensor.matmul(
                        apv[:], lhsT=avg[0:10, h * HD:(h + 1) * HD],
                        rhs=apb[:], start=True, stop=True)
                    # normalize + combine (sums already on all partitions)
                    recM = rcpool.tile([128, QW], F32, tag="recM",
                                       name=f"rM{qb}_{h}")
                    nc.vector.reciprocal_approx_fast(recM[:], sums)
                    recA = rcpool.tile([128, QW], F32, tag="recA",
                                       name=f"rA{qb}_{h}")
                    nc.vector.reciprocal_approx_fast(recA[:], sumA)
                    c1 = ctpool.tile([128, QW], F16, tag="c1",
                                     name=f"c1{qb}_{h}")
                    nc.vector.tensor_tensor(c1[:], pv[:], recM[:], op.mult)
                    c2 = ctpool.tile([128, QW], F16, tag="c2",
                                     name=f"c2{qb}_{h}")
                    nc.vector.tensor_tensor(c2[:], apv[:], recA[:],
                                            op.mult)
                    c3 = ctpool.tile([128, QW], F16, tag="c3",
                                     name=f"c3{qb}_{h}")
                    nc.vector.tensor_tensor(c3[:], c1[:], c2[:], op.add)
                    # c3 store on the gpsimd queue: it precedes the
                    # collective in queue order, so the gather launches the
                    # moment the last half is written
                    nc.gpsimd.dma_start(
                        agin[qb][h * 128:(h + 1) * 128, :], c3[:])
                    if qb == 0:
                        # last block: per-head gathers so the drain only
                        # ever waits on a small final collective
                        nc.gpsimd.collective_compute(
                            "AllGather",
                            op.bypass,
                            replica_groups=REPLICA_GROUPS,
                            ins=[agin[0][h * 128:(h + 1) * 128, :].opt()],
                            outs=[ago0[h][:].opt()],
                        )
                    elif h == 3 or h == 7:
                        # half-gather as soon as 4 heads are done so the
                        # collective overlaps the rest of the block
                        hf = h // 4
                        nc.gpsimd.collective_compute(
                            "AllGather",
                            op.bypass,
                            replica_groups=REPLICA_GROUPS,
                            ins=[agin[qb][hf * 512:(hf + 1) * 512,
                                          :].opt()],
                            outs=[agouth[qb][hf][:].opt()],
                        )

                def wo_unit(qb, od):
                    """One od-block of out[od, q] = wo[od, :] @ attn[:, q]:
                    a dense 32-matmul burst that keeps the PE warm between
                    exp-paced attention heads.  wo weights stream as
                    [128,512] slabs (4 contraction chunks per descriptor)
                    so the Sync queue stays far ahead of the PE."""
                    if qb == 0:
                        def ag_ap(i):
                            ci2, j = divmod(i, 8)
                            sl, hl = divmod(j, 4)
                            b = hl * 4 + ci2
                            return agq0_slabs[sl][:, b * QW:(b + 1) * QW]
                    else:
                        slabs = ag_slabs[qb]

                        def ag_ap(i):
                            ci2, j = divmod(i, 8)
                            hf, jl = divmod(j, 4)
                            b = ci2 * 4 + jl
                            return slabs[hf][:, b * QW:(b + 1) * QW]

                    ps = pwop.tile([128, QW], F32, tag="pwo",
                                   name=f"pwo{qb}_{od}")
                    w2s = None
                    for i in range(NM):
                        ig, c = divmod(i, 4)
                        if c == 0:
                            w2s = w2pool.tile([128, 512], F16, tag="w2",
                                              name=f"w2_{qb}_{od}_{ig}")
                            nc.sync.dma_start(
                                w2s[:].rearrange("p (c o) -> p c o", c=4),
                                woT[ig * 512:(ig + 1) * 512,
                                    od * 128:(od + 1) * 128].rearrange(
                                    "(c p) o -> p c o", p=128),
                            )
                        nc.tensor.matmul(
                            ps[:], lhsT=w2s[:, c * 128:(c + 1) * 128],
                            rhs=ag_ap(i),
                            start=(i == 0), stop=(i == NM - 1),
                        )
                    st = ostpool.tile([128, QW], F16, tag="ost",
                                      name=f"st{qb}_{od}")
                    nc.vector.tensor_copy(st[:], ps[:])
                    nc.sync.dma_start(
                        out_ext[od * 128:(od + 1) * 128,
                                qb * QW:(qb + 1) * QW], st[:])

                # weave: pv_part(slot) runs one head behind scores_part;
                # wo units (dense) fill between heads; slabs stage on the
                # gpsimd queue only at points where their gather is done,
                # so the Sync queue (w2 + q feeds) never blocks.
                wo_sched = {}
                for k in range(8):
                    src = (2, 7) if k == 0 else (1, k - 1)
                    wo_sched[src] = (3, k)
                for k in range(8):
                    src = (1, 7) if k == 0 else (0, k - 1)
                    wo_sched[src] = (2, k)
                wo_sched[(0, 7)] = (1, 0)

                def hooks(qb, h):
                    if (qb, h) in wo_sched:
                        wo_unit(*wo_sched[(qb, h)])
                    if qb == 0 and h in (2, 3):
                        stage_agq0(h - 2)
                    if qb in (1, 2) and h == 4:
                        stage_ag_half(qb + 1, 0)
                    if qb in (1, 2) and h == 5:
                        stage_ag_half(qb + 1, 1)
                    if qb == 0 and h == 4:
                        stage_agq0(2)
                        stage_ag_half(1, 0)
                    if qb == 0 and h == 5:
                        stage_agq0(3)
                        stage_ag_half(1, 1)

                seq = [(qb, h) for qb in range(QB - 1, -1, -1)
                       for h in range(HPC)]
                for idx, (qb, h) in enumerate(seq):
                    scores_part(qb, h)
                    if idx > 0:
                        pqb, ph = seq[idx - 1]
                        pv_part(pqb, ph)
                        hooks(pqb, ph)
                pv_part(0, 7)
                wo_unit(1, 0)
                for h in range(4, HPC):
                    stage_agq0(h)
                    wo_unit(1, h - 3)
                for od in range(5, 8):
                    wo_unit(1, od)
                for od in range(8):
                    wo_unit(0, od)

    nc.compile()
    return nc


# ---------------------------------------------------------------------------
# host-side input prep + execution
# ---------------------------------------------------------------------------

_DEINT = np.concatenate([np.arange(0, 128, 2), np.arange(1, 128, 2)])


def _prep_inputs(x, adapter, wq, wk, wv, wo, gate, freqs_cos, freqs_sin, mask):
    """Build the per-core input maps."""
    perm = np.concatenate([h * HD + _DEINT for h in range(H)])  # deinterleave
    wqp = wq[perm, :]  # permute output dims of wq/wk for rope layout
    wkp = wk[perm, :]

    in_maps = []
    for c in range(NCORES):
        g, ci = divmod(c, CPG)
        osl = slice(ci * OC, (ci + 1) * OC)
        xT = np.zeros((DIM, TAUG), FP16)
        xT[:, :S] = x[g].T.astype(FP16)
        xT[:, S:S + ALEN] = adapter[0].T.astype(FP16)
        # diagonal-block masks are invariant to the query-block offset
        mm = np.empty((4, 128, QW), FP16)
        for dk in range(4):
            k0 = dk * 128
            mm[dk] = np.exp(
                mask[0, 0, 0:QW, k0:k0 + 128]).T.astype(FP16)
        gates = np.zeros((16, HPC), np.float32)
        gates[:, :] = gate[0, ci * HPC:(ci + 1) * HPC, 0, 0][None, :]
        in_maps.append({
            "xT": xT,
            "wqT": np.ascontiguousarray(wqp[osl].T).astype(FP16),
            "wkT": np.ascontiguousarray(wkp[osl].T).astype(FP16),
            "wvT": np.ascontiguousarray(wv[osl].T).astype(FP16),
            "woT": np.ascontiguousarray(wo[osl].T).astype(FP16),
            "cosP": freqs_cos.astype(FP16),
            "sinP": freqs_sin.astype(FP16),
            "maskmul": mm,
            "gates": gates,
            "eye": np.eye(128, dtype=FP16),
        })
    return in_maps


_NC_CACHE = {}
TRACE = bool(int(os.environ.get("BASS_KERNEL_TRACE", "0")))
LAST_EXEC_NS = None
LAST_RESULTS = None


def kernel(x, adapter, wq, wk, wv, wo, gate, freqs_cos, freqs_sin, mask,
           start_pos=0, **_unused):
    global LAST_EXEC_NS, LAST_RESULTS
    from concourse.bass_utils import run_bass_kernel_spmd

    to_np = lambda a: np.asarray(a)
    x, adapter, wq, wk, wv, wo = map(to_np, (x, adapter, wq, wk, wv, wo))
    gate, freqs_cos, freqs_sin, mask = map(
        to_np, (gate, freqs_cos, freqs_sin, mask))

    if "nc" not in _NC_CACHE:
        _NC_CACHE["nc"] = build_graph()
    nc = _NC_CACHE["nc"]

    in_maps = _prep_inputs(x, adapter, wq, wk, wv, wo, gate,
                           freqs_cos, freqs_sin, mask)
    res = run_bass_kernel_spmd(
        nc, in_maps, core_ids=list(range(NCORES)), trace=TRACE)
    LAST_EXEC_NS = res.exec_time_ns
    LAST_RESULTS = res
    out = np.empty((B, S, DIM), np.float32)
    for c in range(NCORES):
        g, ci = divmod(c, CPG)
        out[g, :, ci * OC:(ci + 1) * OC] = res.results[c]["out"].T
    return out



# revision 30
# speedup vs baseline: 1.0472x; 1.0472x over previous
"""Trainium2 Bass kernel for gated-adapter attention (Llama-Adapter style).

Sharding: 2 data-parallel groups of 4 cores (batch dim); within a group the 32
heads are tensor-parallel (8 heads/core).  Each core computes QKV + RoPE +
causal flash-style attention (transposed scores) + gated adapter cross
attention for its heads, AllGathers the per-head attention output across its
group of 4, then computes a column shard of the wo projection.  The host
reassembles the full [2, 2048, 4096] output from the 8 per-core shards.

Structure (v2):
 - Phase A: two head-half passes; each pass computes Q,K,V for 4 heads in one
   dense matmul stream (3 N=512 matmuls per (tchunk, m) sharing the x
   stationary), so the PE never idles at projection boundaries and the HAM
   clock stays warm.  RoPE + per-head PE transposes produce qT/kT layouts.
 - Phase B: attention blocks and wo chunks interleaved in one scope so wo
   matmuls fill softmax-latency gaps; AllGather(qb) overlaps attention(qb-1).

All TensorEngine tensors are fp16 (1 cycle/row, ~0.05% rounding); PSUM
accumulation is fp32; softmax sums/reciprocals are fp32.
"""

import math
import os
import sys

import numpy as np

for _p in ("/opt/trn_rl_repo",):
    if os.path.isdir(_p) and _p not in sys.path:
        sys.path.insert(0, _p)

import ml_dtypes  # noqa: E402

import concourse.bass as bass  # noqa: E402
import concourse.mybir as mybir  # noqa: E402
import concourse.tile as tile  # noqa: E402
from concourse import bacc  # noqa: E402

FP16 = np.float16
F16 = mybir.dt.float16
F32 = mybir.dt.float32

DIM = 4096
S = 2048
B = 2
H = 32
HD = 128
ALEN = 10

NCORES = 8
CPG = 4          # cores per group (group = one batch element)
HPC = 8          # heads per core
OC = HPC * HD    # 1024 output dims per core for q/k/v and for wo columns
HHP = 4          # heads per pass
OCH = HHP * HD   # 512 output dims per pass

TCN = 17         # t-chunks of 128: 16 real + 1 adapter/pad chunk
TAUG = TCN * 128  # 2176
NM = DIM // 128   # 32 contraction chunks
QB = 4           # query blocks
QW = 512         # query block width
SCALE = 1.0 / math.sqrt(HD)

REPLICA_GROUPS = [[0, 1, 2, 3], [4, 5, 6, 7]]

Exp = mybir.ActivationFunctionType.Exp
Copy = mybir.ActivationFunctionType.Copy


def _alu():
    from concourse.alu_op_type import AluOpType
    return AluOpType


def build_graph():
    nc = bacc.Bacc(
        "TRN2",
        target_bir_lowering=False,
        debug=False,
        num_devices=NCORES,
    )

    # ---- external I/O ------------------------------------------------------
    xT = nc.dram_tensor("xT", [DIM, TAUG], F16, kind="ExternalInput")
    wqT = nc.dram_tensor("wqT", [DIM, OC], F16, kind="ExternalInput")
    wkT = nc.dram_tensor("wkT", [DIM, OC], F16, kind="ExternalInput")
    wvT = nc.dram_tensor("wvT", [DIM, OC], F16, kind="ExternalInput")
    woT = nc.dram_tensor("woT", [DIM, OC], F16, kind="ExternalInput")
    cosP = nc.dram_tensor("cosP", [S, 64], F16, kind="ExternalInput")
    sinP = nc.dram_tensor("sinP", [S, 64], F16, kind="ExternalInput")
    maskmul = nc.dram_tensor("maskmul", [4, 128, QW], F16, kind="ExternalInput")
    gates = nc.dram_tensor("gates", [16, HPC], F32, kind="ExternalInput")
    eye = nc.dram_tensor("eye", [128, 128], F16, kind="ExternalInput")
    out_ext = nc.dram_tensor("out", [OC, S], F16, kind="ExternalOutput")

    op = _alu()

    with tile.TileContext(nc) as tc:
        with (
            tc.tile_pool(name="persist", bufs=1) as persist,
            tc.tile_pool(name="dram", bufs=1, space="DRAM") as dpool,
        ):
            kT = persist.tile([128, HPC * S], F16, tag="kT")     # [d, h*t]
            vsb = persist.tile([128, TCN * OC], F16, tag="vsb")  # [t, tc*o]
            akT = persist.tile([128, HPC * 16], F16, tag="akT")  # [d, h*16]
            avg = persist.tile([16, HPC * HD], F16, tag="avg")   # [a, h*d]
            ones = persist.tile([128, 128], F16, tag="ones")
            eyesb = persist.tile([128, 128], F16, tag="eyesb")
            gatesb = persist.tile([16, HPC], F32, tag="gatesb")
            cossb = persist.tile([128, 16 * 64], F16, tag="cossb")
            sinsb = persist.tile([128, 16 * 64], F16, tag="sinsb")
            negC = persist.tile([128, 1], F32, tag="negC")
            # diagonal-block exp masks are the same for every query block
            # (k0 - q0 is fixed per dk), so 4 tiles cover all of them
            mask4 = persist.tile([128, 4 * QW], F16, tag="mask4")

            nc.gpsimd.memset(ones[:], 1.0)
            nc.gpsimd.memset(negC[:], -9.0)
            nc.sync.dma_start(eyesb[:], eye[:])
            nc.sync.dma_start(gatesb[:], gates[:])
            nc.sync.dma_start(
                mask4[:].rearrange("p (d q) -> p d q", d=4),
                maskmul[:].rearrange("d p q -> p d q"),
            )
            nc.sync.dma_start(
                cossb[:].rearrange("p (c d) -> p c d", c=16),
                cosP[:].rearrange("(c p) d -> p c d", p=128),
            )
            nc.sync.dma_start(
                sinsb[:].rearrange("p (c d) -> p c d", c=16),
                sinP[:].rearrange("(c p) d -> p c d", p=128),
            )

            qTd = [dpool.tile([128, HPC * QW], F16, tag=f"qTd{q}",
                              name=f"qTd{q}")
                   for q in range(QB)]
            agin = [dpool.tile([OC, QW], F16, tag=f"agin{q}", name=f"agin{q}")
                    for q in range(QB)]
            agouth = [[dpool.tile([CPG * 512, QW], F16, tag=f"ago{q}_{hf}",
                                  name=f"ago{q}_{hf}")
                       for hf in range(2)]
                      for q in range(QB)]
            # qb=0 gathers per-head so the tail never waits a big collective
            ago0 = [dpool.tile([CPG * 128, QW], F16, tag=f"ago0h{h}",
                               name=f"ago0h{h}")
                    for h in range(HPC)]

            # warm up the CC engine during Phase A: the first collective
            # pays ~25us of one-time setup that would otherwise land on
            # the critical path of the first attention block's wo units
            ccw_in = dpool.tile([16, 8], F32, tag="ccwi", name="ccwi")
            ccw_out = dpool.tile([64, 8], F32, tag="ccwo", name="ccwo")
            nc.gpsimd.dma_start(ccw_in[:], gatesb[:])
            nc.gpsimd.collective_compute(
                "AllGather",
                op.bypass,
                replica_groups=REPLICA_GROUPS,
                ins=[ccw_in[:].opt()],
                outs=[ccw_out[:].opt()],
            )

            # ============ Phase A: QKV + RoPE, two head-half passes ========
            with (
                tc.tile_pool(name="wres", bufs=96) as wpool,
                tc.tile_pool(name="xin", bufs=30) as xpool,
                tc.tile_pool(name="xin8", bufs=6) as xpool8,
                tc.tile_pool(name="asm", bufs=5) as apool,
                tc.tile_pool(name="rot", bufs=5) as rpool,
                tc.tile_pool(name="qst", bufs=3) as qstpool,
                tc.tile_pool(name="psq", bufs=2, space="PSUM") as psqp,
                tc.tile_pool(name="psk", bufs=2, space="PSUM") as pskp,
                tc.tile_pool(name="psv", bufs=2, space="PSUM") as psvp,
                tc.tile_pool(name="pstr", bufs=2, space="PSUM") as ptpool,
            ):
                pending = []   # deferred PE transposes + copies

                def rope(proj, hp, tch, asmt):
                    """asmt: [128 tokens, OCH] = 4 heads, deinterleaved rope
                    pairs.  Rotate now (DVE); defer the PE transposes so the
                    PE queue never stalls on the rope pipeline."""
                    a3 = asmt[:].rearrange("p (h d) -> p h d", h=HHP)[:, :, 0:64]
                    b3 = asmt[:].rearrange("p (h d) -> p h d", h=HHP)[:, :, 64:128]
                    cos1 = cossb[:].rearrange(
                        "p (c o d) -> p c o d", c=16, o=1)[:, tch]
                    sin1 = sinsb[:].rearrange(
                        "p (c o d) -> p c o d", c=16, o=1)[:, tch]
                    cos3, _ = bass.broadcast_tensor_aps(cos1, a3)
                    sin3, _ = bass.broadcast_tensor_aps(sin1, a3)
                    rot = rpool.tile([128, OCH], F16, tag="rot",
                                     name=f"rot{proj}{hp}_{tch}")
                    ra = rot[:].rearrange("p (h d) -> p h d", h=HHP)[:, :, 0:64]
                    rb = rot[:].rearrange("p (h d) -> p h d", h=HHP)[:, :, 64:128]
                    t1 = rpool.tile([128, HHP * 64], F16, tag="rt1",
                                    name=f"rt1{proj}{hp}_{tch}")
                    t13 = t1[:].rearrange("p (h d) -> p h d", h=HHP)
                    t2 = rpool.tile([128, HHP * 64], F16, tag="rt2",
                                    name=f"rt2{proj}{hp}_{tch}")
                    t23 = t2[:].rearrange("p (h d) -> p h d", h=HHP)
                    nc.vector.tensor_tensor(t13, a3, cos3, op.mult)
                    nc.vector.tensor_tensor(t23, b3, sin3, op.mult)
                    nc.vector.tensor_tensor(ra, t13, t23, op.subtract)
                    nc.vector.tensor_tensor(t13, a3, sin3, op.mult)
                    nc.vector.tensor_tensor(t23, b3, cos3, op.mult)
                    nc.vector.tensor_tensor(rb, t13, t23, op.add)
                    pending.append((proj, hp, tch, rot))

                def flush_pending():
                    for proj, hp, tch, rot in pending:
                        ptr = ptpool.tile([128, OCH], F16, tag="pstr",
                                          name=f"ptr{proj}{hp}_{tch}")
                        for hl in range(HHP):
                            nc.tensor.transpose(
                                ptr[:, hl * 128:(hl + 1) * 128],
                                rot[:, hl * 128:(hl + 1) * 128],
                                eyesb[:],
                            )
                        h0 = hp * HHP
                        if proj == "ak":
                            nc.scalar.activation(
                                akT[:].rearrange(
                                    "p (h a) -> p h a",
                                    h=HPC)[:, h0:h0 + HHP, :],
                                ptr[:].rearrange(
                                    "p (h t) -> p h t", h=HHP)[:, :, 0:16],
                                Copy,
                            )
                        elif proj == "k":
                            nc.scalar.activation(
                                kT[:].rearrange(
                                    "p (h t) -> p h t",
                                    h=HPC)[:, h0:h0 + HHP,
                                           tch * 128:(tch + 1) * 128],
                                ptr[:].rearrange("p (h d) -> p h d", h=HHP),
                                Copy,
                            )
                        else:
                            qst = qstpool.tile([128, OCH], F16, tag="qst",
                                               name=f"qst{hp}_{tch}")
                            nc.scalar.activation(qst[:], ptr[:], Copy)
                            qb_of, tl = divmod(tch, 4)
                            nc.scalar.dma_start(
                                qTd[qb_of][:].rearrange(
                                    "p (h t) -> p h t",
                                    h=HPC)[:, h0:h0 + HHP,
                                           tl * 128:(tl + 1) * 128],
                                qst[:].rearrange("p (h d) -> p h d", h=HHP),
                            )
                    pending.clear()

                for hp in range(2):
                    osl = slice(hp * OCH, (hp + 1) * OCH)
                    wres = {"q": [], "k": [], "v": []}

                    # m-major emission so DMA arrival matches the (q,k,v)
                    # per-m consumption order; only 8 groups up front so the
                    # first x tiles aren't queued behind 12MB of weights
                    def load_w_group(m, hp=hp, osl=osl, wres=wres):
                        for proj, wsrc in (("q", wqT), ("k", wkT),
                                           ("v", wvT)):
                            wt = wpool.tile([128, OCH], F16, tag="wres",
                                            name=f"w{proj}{hp}_{m}")
                            nc.sync.dma_start(
                                wt[:], wsrc[m * 128:(m + 1) * 128, osl])
                            wres[proj].append(wt)

                    for m in range(8):
                        load_w_group(m)
                    # x is staged in 2-chunk slabs: tch 16 gets its own
                    # [128,128] tile; even tch prefetches the slab covering
                    # (tch-2, tch-1) a full t-chunk before first use so the
                    # PE never waits on the x feed.
                    xslab = {}
                    # descending: qb=3's q/k chunks finish first so the
                    # first attention block's inputs are ready earliest
                    for tch in range(TCN - 1, -1, -1):
                        psq = None
                        if tch < 16:
                            psq = psqp.tile([128, OCH], F32, tag="psq",
                                            name=f"psq{hp}_{tch}")
                        psk = pskp.tile([128, OCH], F32, tag="psk",
                                        name=f"psk{hp}_{tch}")
                        psv = psvp.tile([128, OCH], F32, tag="psv",
                                        name=f"psv{hp}_{tch}")
                        for m in range(NM):
                            if tch == TCN - 1 and m + 8 < NM:
                                load_w_group(m + 8)
                            if tch == 16:
                                t = xpool8.tile([128, 128], F16, tag="xin8",
                                                name=f"x{hp}_{m}_s8")
                                nc.sync.dma_start(
                                    t[:], xT[m * 128:(m + 1) * 128,
                                             2048:2176])
                                xslab[m] = t
                            if tch == 16:
                                xm_ap = xslab[m][:]
                            else:
                                off = tch % 2
                                xm_ap = xslab[m][:, off * 128:(off + 1) * 128]
                            st = (m == 0)
                            sp = (m == NM - 1)
                            if tch < 16:
                                nc.tensor.matmul(
                                    psq[:], lhsT=xm_ap, rhs=wres["q"][m][:],
                                    start=st, stop=sp)
                            nc.tensor.matmul(
                                psk[:], lhsT=xm_ap, rhs=wres["k"][m][:],
                                start=st, stop=sp)
                            nc.tensor.matmul(
                                psv[:], lhsT=xm_ap, rhs=wres["v"][m][:],
                                start=st, stop=sp)
                            if tch % 2 == 0 and tch >= 2:
                                t = xpool.tile([128, 256], F16, tag="xin",
                                               name=f"x{hp}_{m}_s{tch//2-1}")
                                nc.sync.dma_start(
                                    t[:], xT[m * 128:(m + 1) * 128,
                                             (tch - 2) * 128:tch * 128])
                                xslab[m] = t
                        # ---- V: evacuate straight to vsb ------------------
                        nc.vector.tensor_copy(
                            vsb[:, tch * OC + hp * OCH:
                                tch * OC + (hp + 1) * OCH], psv[:])
                        if tch == 16:
                            for hl in range(HHP):
                                h = hp * HHP + hl
                                nc.vector.tensor_scalar(
                                    avg[0:10, h * HD:(h + 1) * HD],
                                    vsb[0:10, 16 * OC + h * HD:
                                        16 * OC + (h + 1) * HD],
                                    gatesb[0:10, h:h + 1],
                                    None,
                                    op.mult,
                                )
                        # ---- K (+ Q): evacuate, rope; transposes deferred -
                        asmk = apool.tile([128, OCH], F16, tag="asm",
                                          name=f"ask{hp}_{tch}")
                        nc.scalar.activation(asmk[:], psk[:], Copy)
                        asmq = None
                        if tch < 16:
                            asmq = apool.tile([128, OCH], F16, tag="asm",
                                              name=f"asq{hp}_{tch}")
                            nc.scalar.activation(asmq[:], psq[:], Copy)
                        flush_pending()
                        if tch < 16:
                            rope("k", hp, tch, asmk)
                            rope("q", hp, tch, asmq)
                        else:
                            pending.append(("ak", hp, tch, asmk))
                flush_pending()

            # ====== Phase B: attention + AllGather + wo, interleaved =======
            from contextlib import ExitStack
            _es = ExitStack()
            with _es:
                P = lambda **kw: _es.enter_context(tc.tile_pool(**kw))
                prpool = P(name="probs", bufs=22)
                qldpool = P(name="qld", bufs=2)
                smpool = P(name="small", bufs=3)
                rcpool = P(name="rec", bufs=2)
                ctpool = P(name="ctmp", bufs=2)
                w2pool = P(name="w2", bufs=8)
                agpool = P(name="agsb", bufs=4)
                ostpool = P(name="ost", bufs=2)
                pscp = P(name="psc", bufs=4, space="PSUM")
                ppvp = P(name="ppv", bufs=2, space="PSUM")
                psmp = P(name="psums", bufs=1, space="PSUM")
                pwop = P(name="pwo", bufs=1, space="PSUM")

                qslabs = {}

                def load_qslab(qb):
                    qt = qldpool.tile([128, HPC * QW], F16, tag="qld",
                                      name=f"qs{qb}")
                    nc.sync.dma_start(qt[:], qTd[qb][:])
                    qslabs[qb] = qt

                load_qslab(QB - 1)

                # gathered-activation staging: each collective half is
                # reloaded as four 512KB pieces so no single DMA blocks the
                # Sync queue long enough to starve the w2 feed
                ag_slabs = {}
                agq0_slabs = {}

                def stage_ag_half(qb, hf):
                    # on the gpsimd queue: never blocks the w2/q feeds,
                    # and is only emitted at points where its gather has
                    # provably completed
                    slab = agpool.tile([128, 16 * QW], F16, tag="agsb",
                                       name=f"ags{qb}_{hf}")
                    ag_slabs.setdefault(qb, {})[hf] = slab
                    nc.gpsimd.dma_start(
                        slab[:].rearrange("p (b q) -> p b q", b=16),
                        agouth[qb][hf][:].rearrange(
                            "(b p) q -> p b q", p=128),
                    )

                def stage_agq0(h):
                    sl = h // 4
                    if sl not in agq0_slabs:
                        agq0_slabs[sl] = agpool.tile(
                            [128, 16 * QW], F16, tag="agsb",
                            name=f"agq0s{sl}")
                    slab = agq0_slabs[sl]
                    hl = h % 4
                    nc.gpsimd.dma_start(
                        slab[:, hl * 4 * QW:(hl + 1) * 4 * QW].rearrange(
                            "p (c q) -> p c q", c=CPG),
                        ago0[h][:].rearrange("(c p) q -> p c q", p=128),
                    )

                pend = {}

                def scores_part(qb, h):
                    """scores + exp (+ adapter scores) for one head.  Runs
                    one head ahead of pv_part so the Act engine's exp
                    latency hides under the previous head's pv stream."""
                    kk = (qb + 1) * 4  # causal: k chunks 0..kk-1
                    if h == 0 and qb > 0:
                        load_qslab(qb - 1)  # prefetch next block's q
                    q_ap = qslabs[qb][:, h * QW:(h + 1) * QW]
                    pbs = []
                    for kc in range(kk):
                        sc = pscp.tile([128, QW], F32, tag="sc",
                                       name=f"sc{qb}_{h}_{kc}")
                        nc.tensor.matmul(
                            sc[:],
                            lhsT=kT[:, h * S + kc * 128:
                                    h * S + (kc + 1) * 128],
                            rhs=q_ap,
                            start=True, stop=True,
                        )
                        pb = prpool.tile([128, QW], F16, tag="probs",
                                         name=f"pb{qb}_{h}_{kc}")
                        nc.scalar.activation(pb[:], sc[:], Exp,
                                             bias=negC[:, 0:1],
                                             scale=SCALE)
                        if kc >= qb * 4:
                            dk = kc - qb * 4
                            nc.vector.tensor_tensor(
                                pb[:], pb[:],
                                mask4[:, dk * QW:(dk + 1) * QW], op.mult)
                        pbs.append(pb)
                    asc = pscp.tile([10, QW], F32, tag="sc",
                                    name=f"asc{qb}_{h}")
                    nc.tensor.matmul(
                        asc[:], lhsT=akT[:, h * 16:h * 16 + 10],
                        rhs=q_ap, start=True, stop=True)
                    apb = smpool.tile([10, QW], F16, tag="aprobs",
                                      name=f"apb{qb}_{h}")
                    nc.scalar.activation(apb[:], asc[:], Exp,
                                         bias=negC[0:10, 0:1],
                                         scale=SCALE)
                    pend[(qb, h)] = (pbs, apb)

                def pv_part(qb, h):
                    kk = (qb + 1) * 4
                    pbs, apb = pend.pop((qb, h))
                    # sums matmuls use a full ones[128,128] stationary, so
                    # the row-sum lands replicated on all 128 partitions --
                    # no gpsimd partition_broadcast needed downstream.
                    smt = psmp.tile([128, QW], F32, tag="sums",
                                    name=f"sums{qb}_{h}")
                    sums = smt[:]
                    pv = ppvp.tile([128, QW], F32, tag="pv",
                                   name=f"pv{qb}_{h}")
                    for i in range(kk):
                        nc.tensor.matmul(
                            sums, lhsT=ones[:], rhs=pbs[i][:],
                            start=(i == 0), stop=(i == kk - 1),
                        )
                        nc.tensor.matmul(
                            pv[:],
                            lhsT=vsb[:, i * OC + h * HD:
                                     i * OC + (h + 1) * HD],
                            rhs=pbs[i][:],
                            start=(i == 0), stop=(i == kk - 1),
                        )
                    sat = pscp.tile([128, QW], F32, tag="sc",
                                    name=f"sumA{qb}_{h}")
                    sumA = sat[:]
                    nc.tensor.matmul(
                        sumA, lhsT=ones[0:10, :], rhs=apb[:],
                        start=True, stop=True)
                    apv = ppvp.tile([128, QW], F32, tag="pv",
                                    name=f"apv{qb}_{h}")
                    nc.tensor.matmul(
                        apv[:], lhsT=avg[0:10, h * HD:(h + 1) * HD],
                        rhs=apb[:], start=True, stop=True)
                    # normalize + combine (sums already on all partitions)
                    recM = rcpool.tile([128, QW], F32, tag="recM",
                                       name=f"rM{qb}_{h}")
                    nc.vector.reciprocal_approx_fast(recM[:], sums)
                    recA = rcpool.tile([128, QW], F32, tag="recA",
                                       name=f"rA{qb}_{h}")
                    nc.vector.reciprocal_approx_fast(recA[:], sumA)
                    c1 = ctpool.tile([128, QW], F16, tag="c1",
                                     name=f"c1{qb}_{h}")
                    nc.vector.tensor_tensor(c1[:], pv[:], recM[:], op.mult)
                    c2 = ctpool.tile([128, QW], F16, tag="c2",
                                     name=f"c2{qb}_{h}")
                    nc.vector.tensor_tensor(c2[:], apv[:], recA[:],
                                            op.mult)
                    c3 = ctpool.tile([128, QW], F16, tag="c3",
                                     name=f"c3{qb}_{h}")
                    nc.vector.tensor_tensor(c3[:], c1[:], c2[:], op.add)
                    # c3 store on the gpsimd queue: it precedes the
                    # collective in queue order, so the gather launches the
                    # moment the last half is written
                    nc.gpsimd.dma_start(
                        agin[qb][h * 128:(h + 1) * 128, :], c3[:])
                    if qb == 0:
                        # last block: per-head gathers so the drain only
                        # ever waits on a small final collective
                        nc.gpsimd.collective_compute(
                            "AllGather",
                            op.bypass,
                            replica_groups=REPLICA_GROUPS,
                            ins=[agin[0][h * 128:(h + 1) * 128, :].opt()],
                            outs=[ago0[h][:].opt()],
                        )
                    elif h == 3 or h == 7:
                        # half-gather as soon as 4 heads are done so the
                        # collective overlaps the rest of the block
                        hf = h // 4
                        nc.gpsimd.collective_compute(
                            "AllGather",
                            op.bypass,
                            replica_groups=REPLICA_GROUPS,
                            ins=[agin[qb][hf * 512:(hf + 1) * 512,
                                          :].opt()],
                            outs=[agouth[qb][hf][:].opt()],
                        )

                def wo_unit(qb, od):
                    """One od-block of out[od, q] = wo[od, :] @ attn[:, q]:
                    a dense 32-matmul burst that keeps the PE warm between
                    exp-paced attention heads.  wo weights stream as
                    [128,512] slabs (4 contraction chunks per descriptor)
                    so the Sync queue stays far ahead of the PE."""
                    if qb == 0:
                        def ag_ap(i):
                            ci2, j = divmod(i, 8)
                            sl, hl = divmod(j, 4)
                            b = hl * 4 + ci2
                            return agq0_slabs[sl][:, b * QW:(b + 1) * QW]
                    else:
                        slabs = ag_slabs[qb]

                        def ag_ap(i):
                            ci2, j = divmod(i, 8)
                            hf, jl = divmod(j, 4)
                            b = ci2 * 4 + jl
                            return slabs[hf][:, b * QW:(b + 1) * QW]

                    ps = pwop.tile([128, QW], F32, tag="pwo",
                                   name=f"pwo{qb}_{od}")
                    w2s = None
                    for i in range(NM):
                        ig, c = divmod(i, 4)
                        if c == 0:
                            w2s = w2pool.tile([128, 512], F16, tag="w2",
                                              name=f"w2_{qb}_{od}_{ig}")
                            nc.sync.dma_start(
                                w2s[:].rearrange("p (c o) -> p c o", c=4),
                                woT[ig * 512:(ig + 1) * 512,
                                    od * 128:(od + 1) * 128].rearrange(
                                    "(c p) o -> p c o", p=128),
                            )
                        nc.tensor.matmul(
                            ps[:], lhsT=w2s[:, c * 128:(c + 1) * 128],
                            rhs=ag_ap(i),
                            start=(i == 0), stop=(i == NM - 1),
                        )
                    st = ostpool.tile([128, QW], F16, tag="ost",
                                      name=f"st{qb}_{od}")
                    nc.vector.tensor_copy(st[:], ps[:])
                    nc.sync.dma_start(
                        out_ext[od * 128:(od + 1) * 128,
                                qb * QW:(qb + 1) * QW], st[:])

                # weave: pv_part(slot) runs one head behind scores_part;
                # wo units (dense) fill between heads; slabs stage on the
                # gpsimd queue only at points where their gather is done,
                # so the Sync queue (w2 + q feeds) never blocks.
                wo_sched = {}
                for k in range(8):
                    src = (2, 7) if k == 0 else (1, k - 1)
                    wo_sched[src] = (3, k)
                for k in range(8):
                    src = (1, 7) if k == 0 else (0, k - 1)
                    wo_sched[src] = (2, k)
                wo_sched[(0, 7)] = (1, 0)

                def hooks(qb, h):
                    if (qb, h) in wo_sched:
                        wo_unit(*wo_sched[(qb, h)])
                    if qb == 0 and h in (2, 3):
                        stage_agq0(h - 2)
                    if qb in (1, 2) and h == 4:
                        stage_ag_half(qb + 1, 0)
                    if qb in (1, 2) and h == 5:
                        stage_ag_half(qb + 1, 1)
                    if qb == 0 and h == 4:
                        stage_agq0(2)
                        stage_ag_half(1, 0)
                    if qb == 0 and h == 5:
                        stage_agq0(3)
                        stage_ag_half(1, 1)

                seq = [(qb, h) for qb in range(QB - 1, -1, -1)
                       for h in range(HPC)]
                for idx, (qb, h) in enumerate(seq):
                    scores_part(qb, h)
                    if idx > 0:
                        pqb, ph = seq[idx - 1]
                        pv_part(pqb, ph)
                        hooks(pqb, ph)
                pv_part(0, 7)
                wo_unit(1, 0)
                for h in range(4, HPC):
                    stage_agq0(h)
                    wo_unit(1, h - 3)
                for od in range(5, 8):
                    wo_unit(1, od)
                for od in range(8):
                    wo_unit(0, od)

    nc.compile()
    return nc


# ---------------------------------------------------------------------------
# host-side input prep + execution
# ---------------------------------------------------------------------------

_DEINT = np.concatenate([np.arange(0, 128, 2), np.arange(1, 128, 2)])


def _prep_inputs(x, adapter, wq, wk, wv, wo, gate, freqs_cos, freqs_sin, mask):
    """Build the per-core input maps."""
    perm = np.concatenate([h * HD + _DEINT for h in range(H)])  # deinterleave
    wqp = wq[perm, :]  # permute output dims of wq/wk for rope layout
    wkp = wk[perm, :]

    in_maps = []
    for c in range(NCORES):
        g, ci = divmod(c, CPG)
        osl = slice(ci * OC, (ci + 1) * OC)
        xT = np.zeros((DIM, TAUG), FP16)
        xT[:, :S] = x[g].T.astype(FP16)
        xT[:, S:S + ALEN] = adapter[0].T.astype(FP16)
        # diagonal-block masks are invariant to the query-block offset
        mm = np.empty((4, 128, QW), FP16)
        for dk in range(4):
            k0 = dk * 128
            mm[dk] = np.exp(
                mask[0, 0, 0:QW, k0:k0 + 128]).T.astype(FP16)
        gates = np.zeros((16, HPC), np.float32)
        gates[:, :] = gate[0, ci * HPC:(ci + 1) * HPC, 0, 0][None, :]
        in_maps.append({
            "xT": xT,
            "wqT": np.ascontiguousarray(wqp[osl].T).astype(FP16),
            "wkT": np.ascontiguousarray(wkp[osl].T).astype(FP16),
            "wvT": np.ascontiguousarray(wv[osl].T).astype(FP16),
            "woT": np.ascontiguousarray(wo[osl].T).astype(FP16),
            "cosP": freqs_cos.astype(FP16),
            "sinP": freqs_sin.astype(FP16),
            "maskmul": mm,
            "gates": gates,
            "eye": np.eye(128, dtype=FP16),
        })
    return in_maps


_NC_CACHE = {}
TRACE = bool(int(os.environ.get("BASS_KERNEL_TRACE", "0")))
LAST_EXEC_NS = None
LAST_RESULTS = None


def kernel(x, adapter, wq, wk, wv, wo, gate, freqs_cos, freqs_sin, mask,
           start_pos=0, **_unused):
    global LAST_EXEC_NS, LAST_RESULTS
    from concourse.bass_utils import run_bass_kernel_spmd

    to_np = lambda a: np.asarray(a)
    x, adapter, wq, wk, wv, wo = map(to_np, (x, adapter, wq, wk, wv, wo))
    gate, freqs_cos, freqs_sin, mask = map(
        to_np, (gate, freqs_cos, freqs_sin, mask))

    if "nc" not in _NC_CACHE:
        _NC_CACHE["nc"] = build_graph()
    nc = _NC_CACHE["nc"]

    in_maps = _prep_inputs(x, adapter, wq, wk, wv, wo, gate,
                           freqs_cos, freqs_sin, mask)
    res = run_bass_kernel_spmd(
        nc, in_maps, core_ids=list(range(NCORES)), trace=TRACE)
    LAST_EXEC_NS = res.exec_time_ns
    LAST_RESULTS = res
    out = np.empty((B, S, DIM), np.float32)
    for c in range(NCORES):
        g, ci = divmod(c, CPG)
        out[g, :, ci * OC:(ci + 1) * OC] = res.results[c]["out"].T
    return out

